# revision 9
# baseline (speedup 1.0000x reference)
"""Trainium2 Bass kernel for nn_BCErrorCNN (dense_cnn).

Network (per sample, input [17, 9]):
  Conv1D(128, k=3, relu) -> [15, 128]   (position 14 dead: never consumed)
  LocallyConnected1D(128, k=3, relu) -> [13, 128]  (position 12 dead)
  MaxPool1D(2) -> [6, 128]
  LocallyConnected1D(128, k=3, relu) -> [4, 128]
  GlobalAvgPool -> [128]; Dense(100, relu); Dense(1, sigmoid)

Sharding: pure data parallelism, batch 32768 -> 8 cores x 4096.

On-chip layout: activations are [feature(partition), batch(free)] in bf16
(fp32 PSUM accumulation). bf16 keeps the PE at 1 col/cycle but enables
fast weight load (LDWEIGHTS fully hidden), 1.0 cyc/row transposes, and
halves all HBM traffic. Each 128-sample group is transposed on the PE
into a single [128, 1024] tile TAB (feature rows 0..127 as cols 0:512,
rows 25..152 as cols 512:1024), evacuated with ONE wide DVE copy. The
conv runs as 14 dense K=128 matmuls against host-side zero-padded
weights wpad, so position l reads TAB directly at the right offset.
lc2's outputs are never materialized: the relu evacuations accumulate
the global-average sum via a scalar_tensor_tensor chain, and d1_w is
pre-scaled by 1/4. Per-tile sigmoid + output DMA keeps the tail off the
critical path. Batch tiles are software-pipelined with a 2-tile skew.

DMA ordering at startup: X0 is issued on the sync queue BEFORE the conv
weights so its descriptors win the (shared) DMA engines; the PE runs
HAM warm-up matmuls until X0 lands so the clock un-throttles early.
"""

import functools

import numpy as np

# ---- constants (hardcoded per problem spec) --------------------------------
N_CORES = 8
B_FULL = 32768
BC = B_FULL // N_CORES  # per-core batch
NB = 512                # batch tile (columns per matmul)
NT = BC // NB           # batch tiles per core
LIN, CIN, F = 17, 9, 128
FEAT = LIN * CIN        # 153
NPOS = 14               # conv positions actually needed (0..13)
NL1 = 12                # lc1 positions needed (0..11)
NPOOL = 6
NL2 = 4
ND1 = 100
NWARM = 16              # HAM warm-up matmuls (N=128 each, ~214ns cold)


def _build_program(nt=NT, flags=(True, True, True, True)):
    import concourse.tile as tile
    from concourse import bacc, mybir
    from concourse.masks import make_identity
    from concourse.tile import add_dep_helper

    cbz, lc1z, lc2z, dbz = flags
    F32 = mybir.dt.float32
    BF16 = mybir.dt.bfloat16
    AF = mybir.ActivationFunctionType
    ALU = mybir.AluOpType

    bc = nt * NB
    nc = bacc.Bacc("TRN2", target_bir_lowering=False, debug=False,
                   num_devices=N_CORES)

    x = nc.dram_tensor("x", [bc * FEAT], BF16, kind="ExternalInput").ap()
    wp = nc.dram_tensor("wp", [F, NPOS * F], BF16, kind="ExternalInput").ap()
    w1 = nc.dram_tensor("w1", [F, NL1 * 3 * F], BF16, kind="ExternalInput").ap()
    w2 = nc.dram_tensor("w2", [F, NL2 * 3 * F], BF16, kind="ExternalInput").ap()
    wd1 = nc.dram_tensor("wd1", [F, ND1], BF16, kind="ExternalInput").ap()
    wd2 = nc.dram_tensor("wd2", [ND1, 1], BF16, kind="ExternalInput").ap()
    cb = nc.dram_tensor("cb", [F, 1], F32, kind="ExternalInput").ap()
    b1 = nc.dram_tensor("b1", [F, NL1], F32, kind="ExternalInput").ap()
    b2 = nc.dram_tensor("b2", [F, NL2], F32, kind="ExternalInput").ap()
    db = nc.dram_tensor("db", [ND1, 1], F32, kind="ExternalInput").ap()
    y = nc.dram_tensor("y", [bc], F32, kind="ExternalOutput").ap()

    with tile.TileContext(nc) as tc:
        with (
            tc.tile_pool(name="const", bufs=1) as cpool,
            tc.tile_pool(name="xg", bufs=3) as xpool,
            tc.tile_pool(name="sg", bufs=2) as spool,
            tc.tile_pool(name="h", bufs=4) as hpool,
            tc.tile_pool(name="m", bufs=2) as mpool,
            tc.tile_pool(name="sa", bufs=8) as sapool,
            tc.tile_pool(name="s3", bufs=2) as s3pool,
            tc.tile_pool(name="yo", bufs=2) as ypool,
            tc.tile_pool(name="psT", bufs=1, space="PSUM") as psT,
            tc.tile_pool(name="psC", bufs=2, space="PSUM") as psC,
            tc.tile_pool(name="psL", bufs=3, space="PSUM") as psL,
        ):
            X_dma = {}

            def load_X(it, queue=None):
                # Batch-permuted layout: partition p holds samples
                # it*512 + p*4 + g (g=0..3), so each partition's source is
                # one contiguous 4*153-elem chunk -> 128 descriptors/tile
                # instead of 512. Xt[p, g*153+f] = x[(it*512+p*4+g)*153+f].
                # The permutation is undone in the sigmoid's output AP.
                Xt = xpool.tile([128, 4 * FEAT], BF16, tag="X", name=f"X{it}")
                base = it * 512 * FEAT
                src = x[base:base + 1].copy()
                src.ap = src.ap[:0] + [[4 * FEAT, 128], [1, 4 * FEAT]]
                X_dma[it] = (queue or nc.sync).dma_start(Xt[:], src)
                return Xt

            # X0 first: its descriptors win the shared DMA engines
            X_pre = {0: load_X(0)}

            # identity for PE transposes + HAM warm-up (GpSimd, no DMA)
            ident = cpool.tile([128, 128], BF16)
            make_identity(nc, ident[:])

            # conv weights enqueued on the same sync queue, split around X0:
            # positions 0-1 first (tiny), the rest after X0, so the first
            # conv matmuls never wait while X0 still wins the DMA engines
            wpt = cpool.tile([128, NPOS * F], BF16)
            nc.sync.dma_start(wpt[:, 0:2 * F], wp[:, 0:2 * F])
            nc.sync.dma_start(wpt[:, 2 * F:], wp[:, 2 * F:])

            if nt > 1:
                X_pre[1] = load_X(1)

            cbt = b1t = b2t = dbt = None
            if not cbz:
                cbt = cpool.tile([F, 1], F32)
                nc.scalar.dma_start(cbt[:], cb[:])

            # HAM warm-up: keep the PE streaming through the X0 DMA wait so
            # the 4096-cycle activity window unthrottles the clock (1.2 ->
            # 2.4 GHz) before real matmuls start. Result is never read.
            pwarm = psL.tile([128, NB], F32, tag="L", name="pwarm")
            for i in range(NWARM):
                nc.tensor.matmul(pwarm[:, 0:128], ident[:], ident[:],
                                 start=(i == 0), stop=(i == NWARM - 1))

            # lc/dense weights host-packed to the exact SBUF layout.
            # Their dma_starts are deferred into produce(0)/produce(1) so
            # the X0/X1 input transfers win the DMA engines at startup.
            w1t = cpool.tile([128, NL1 * 3 * F], BF16)
            w2t = cpool.tile([128, NL2 * 3 * F], BF16)
            wd1t = cpool.tile([128, ND1], BF16)
            wd2t = cpool.tile([ND1, 1], BF16)

            def load_weights_a():
                # big w1 transfer waits for X0 so the input tiles win the
                # DMA engines and the first transposes start early
                half = NL1 * 3 * F // 2
                d = nc.scalar.dma_start(w1t[:, 0:half], w1[:, 0:half])
                add_dep_helper(d.ins, X_dma[0].ins, sync=True,
                               reason="w1 transfer after X0 lands")
                d = nc.scalar.dma_start(w1t[:, half:], w1[:, half:])
                add_dep_helper(d.ins, X_dma[0].ins, sync=True,
                               reason="w1 transfer after X0 lands")
                if not lc1z:
                    nonlocal b1t
                    b1t = cpool.tile([F, NL1], F32)
                    nc.scalar.dma_start(b1t[:], b1[:])

            def load_weights_b():
                nonlocal b2t, dbt
                for dst_t, src_t in ((w2t, w2), (wd1t, wd1), (wd2t, wd2)):
                    d = nc.scalar.dma_start(dst_t[:], src_t[:])
                    add_dep_helper(d.ins, X_dma[1].ins, sync=True,
                                   reason="weight transfer after X1 lands")
                if not lc2z:
                    b2t = cpool.tile([F, NL2], F32)
                    nc.scalar.dma_start(b2t[:], b2[:])
                if not dbz:
                    dbt = cpool.tile([ND1, 1], F32)
                    nc.scalar.dma_start(dbt[:], db[:])

            H_tiles = {}

            def produce(it):
                Xt = X_pre.pop(it) if it in X_pre else load_X(it)
                if it + 2 < nt and it + 2 not in X_pre:
                    X_pre[it + 2] = load_X(it + 2)
                if it == 0:
                    load_weights_a()
                elif it == 1:
                    load_weights_b()

                # ---- transposes into one [128, 1024] tile: TA cols 0:512
                # (feature rows 0..127), TB cols 512:1024 (rows 25..152);
                # single wide evacuation copy on DVE
                TAB = spool.tile([128, 2 * NB], BF16, tag="TAB",
                                 name=f"TAB{it}")
                pT = psT.tile([128, 2 * NB], BF16, tag="T")
                for h, off in ((0, 0), (1, 25)):
                    for g in range(4):
                        nc.tensor.transpose(
                            pT[:, h * NB + g * 128:h * NB + (g + 1) * 128],
                            Xt[:, g * FEAT + off:g * FEAT + off + 128],
                            ident[:])
                nc.vector.tensor_copy(TAB[:], pT[:])

                # ---- conv: 14 positions, dense K=128 vs zero-padded w --
                H = hpool.tile([128, NPOS * NB], BF16, tag="H", name=f"H{it}")
                for a in range(NPOS // 2):
                    pC = psC.tile([128, 1024], F32, tag="C",
                                  name=f"pC{it}_{a}")
                    for d in range(2):
                        p = 2 * a + d
                        mshift = 0 if p <= 11 else NB
                        nc.tensor.matmul(
                            pC[:, d * NB:(d + 1) * NB],
                            wpt[:, p * F:(p + 1) * F],
                            TAB[:, mshift:mshift + NB],
                            start=True, stop=True)
                    # evac split into halves across ACT and DVE so each
                    # psC pair drains in ~0.7us instead of ~1.4us serial.
                    # During pipeline fill (tiles 0-1) DVE has no lc/pool
                    # work yet, so split 7/7 there instead of 9/5.
                    for d in range(2):
                        hdst = H[:, (2 * a + d) * NB:(2 * a + d + 1) * NB]
                        psrc = pC[:, d * NB:(d + 1) * NB]
                        if it <= 1:
                            on_dve = (d == 1)
                        else:
                            on_dve = (a, d) in ((0, 1), (1, 1), (2, 1),
                                                (3, 1), (4, 1), (6, 1))
                        if on_dve and cbz:
                            nc.vector.tensor_scalar_max(hdst, psrc, 0.0)
                        elif on_dve:
                            nc.vector.tensor_scalar(
                                hdst, psrc, cbt[:], 0.0,
                                op0=ALU.add, op1=ALU.max)
                        else:
                            nc.scalar.activation(
                                hdst, psrc, AF.Relu,
                                bias=0.0 if cbz else cbt[:])
                H_tiles[it] = H

            Sacc_tiles = {}

            def consume_lc(it):
                H = H_tiles.pop(it)
                # ---- lc1 (12 positions) + fused maxpool+relu ----------
                # max is associative: max(relu(a), b) == relu(max(a, b)),
                # so evac even psum to E, then max(E, odd psum) on DVE.
                M = mpool.tile([128, NPOOL * NB], BF16, tag="M")
                for t in range(NPOOL):
                    pair = []
                    E = spool.tile([128, NB], BF16, tag="E", name=f"E{it}_{t}")
                    for d in range(2):
                        l = 2 * t + d
                        ps = psL.tile([128, NB], F32, tag="L")
                        for k in range(3):
                            nc.tensor.matmul(
                                ps[:],
                                w1t[:, (l * 3 + k) * F:(l * 3 + k + 1) * F],
                                H[:, (l + k) * NB:(l + k + 1) * NB],
                                start=(k == 0), stop=(k == 2))
                        pair.append(ps)
                        if d == 0:
                            # evac even psum immediately (t=0 on DVE for
                            # engine balance, rest on ACT)
                            if t == 0 and lc1z:
                                nc.vector.tensor_scalar_max(E[:], ps[:], 0.0)
                            elif t == 0:
                                nc.vector.tensor_scalar(
                                    E[:], ps[:], b1t[:, 2 * t:2 * t + 1],
                                    0.0, op0=ALU.add, op1=ALU.max)
                            else:
                                bias = (0.0 if lc1z
                                        else b1t[:, 2 * t:2 * t + 1])
                                nc.scalar.activation(E[:], ps[:], AF.Relu,
                                                     bias=bias)
                    mdst = M[:, t * NB:(t + 1) * NB]
                    if lc1z:
                        nc.vector.tensor_tensor(mdst, E[:], pair[1][:],
                                                op=ALU.max)
                    else:
                        nc.vector.scalar_tensor_tensor(
                            mdst, pair[1][:], b1t[:, 2 * t + 1:2 * t + 2],
                            E[:], op0=ALU.add, op1=ALU.max)

                # ---- lc2 (4 positions), fused with global-avg accum ---
                Sacc = None
                for l in range(NL2):
                    ps = psL.tile([128, NB], F32, tag="L")
                    for k in range(3):
                        nc.tensor.matmul(
                            ps[:],
                            w2t[:, (l * 3 + k) * F:(l * 3 + k + 1) * F],
                            M[:, (l + k) * NB:(l + k + 1) * NB],
                            start=(k == 0), stop=(k == 2))
                    Snew = sapool.tile([128, NB], BF16, tag="SA",
                                       name=f"SA{it}_{l}")
                    sdst = Snew[:]
                    if l == 0:
                        nc.scalar.activation(
                            sdst, ps[:], AF.Relu,
                            bias=0.0 if lc2z else b2t[:, 0:1])
                    elif lc2z:
                        # Snew = relu(ps) + Sacc in one DVE op
                        nc.vector.scalar_tensor_tensor(
                            sdst, ps[:], 0.0, Sacc[:],
                            op0=ALU.max, op1=ALU.add)
                    else:
                        E2 = spool.tile([128, NB], BF16, tag="E2",
                                        name=f"E2{it}_{l}")
                        nc.scalar.activation(E2[:], ps[:], AF.Relu,
                                             bias=b2t[:, l:l + 1])
                        nc.vector.tensor_tensor(sdst, E2[:], Sacc[:],
                                                op=ALU.add)
                    Sacc = Snew
                Sacc_tiles[it] = Sacc

            S3_tiles = {}

            def consume_d1(it):
                # ---- dense1 (wd1 pre-scaled by 1/4): runs a tile behind
                # consume_lc AND before it in emission order, so its psL
                # buffer was last read a full iteration ago and the lc2
                # accum chain (serial DVE ops) never gates the matmul
                Sacc = Sacc_tiles.pop(it)
                pD1 = psL.tile([128, NB], F32, tag="L")
                nc.tensor.matmul(pD1[0:ND1, :], wd1t[:], Sacc[:],
                                 start=True, stop=True)
                S3 = s3pool.tile([ND1, NB], BF16, tag="S3")
                nc.scalar.activation(S3[:], pD1[0:ND1, :],
                                     AF.Relu, bias=0.0 if dbz else dbt[:])
                S3_tiles[it] = S3

            def consume_d2(it):
                # ---- dense2 + sigmoid, one more tile behind so the S3
                # relu (ACT) is long done before the d2 matmul issues
                S3 = S3_tiles.pop(it)
                pD2 = psL.tile([128, NB], F32, tag="L")
                nc.tensor.matmul(pD2[0:1, :], wd2t[:], S3[:],
                                 start=True, stop=True)
                ysb = ypool.tile([1, NB], F32, tag="Y", name=f"Y{it}")
                # undo the batch permutation from load_X: psum column
                # c = g*128 + p -> sample p*4 + g, so write ysb[p*4+g]
                # and the output DMA stays fully contiguous
                ydst = ysb[0:1, 0:1].copy()
                ydst.ap = ydst.ap[:1] + [[1, 4], [4, 128]]
                nc.scalar.activation(ydst, pD2[0:1, :], AF.Sigmoid)
                nc.sync.dma_start(y[it * NB:(it + 1) * NB], ysb[0:1, :])

            # 4-stage pipeline: produce(it) | d1(it-4) | d2(it-5) | lc(it-3).
            # The dense stages are emitted BEFORE consume_lc so their psL
            # buffers were last touched a full iteration earlier; the 3-tile
            # produce->lc skew gives the ACT/DVE evacuation queues a full
            # extra tile of slack before lc1 consumes H.
            for it in range(nt + 5):
                if it < nt:
                    produce(it)
                if 4 <= it < nt + 4:
                    consume_d1(it - 4)
                if it >= 5:
                    consume_d2(it - 5)
                if 3 <= it < nt + 3:
                    consume_lc(it - 3)

    nc.compile()
    return nc


@functools.lru_cache(maxsize=4)
def _get_program(nt, flags):
    return _build_program(nt, flags)


def _prep_in_maps(inputs, conv_w, conv_b, lc1_w, lc1_b, lc2_w, lc2_b,
                  d1_w, d1_b, d2_w, nt=NT, n_cores=N_CORES):
    import ml_dtypes
    bf16 = ml_dtypes.bfloat16
    bc = nt * NB
    f32 = np.float32
    cbz = not np.any(conv_b)
    lc1z = not np.any(lc1_b[:NL1])
    lc2z = not np.any(lc2_b)
    dbz = not np.any(d1_b)
    # conv weights zero-padded to dense K=128 stationaries per position:
    # l<=11 reads TA (feature rows 0..127), l=12,13 read TB (rows 25..152)
    wc = np.asarray(conv_w, dtype=f32).reshape(27, F)
    wp_np = np.zeros((128, NPOS * F), dtype=f32)
    for l in range(NPOS):
        r0 = 9 * l if l <= 11 else 9 * l - 25
        wp_np[r0:r0 + 27, l * F:(l + 1) * F] = wc
    # host-packed to SBUF layout [r(partition), (l k f)]
    w1_np = np.ascontiguousarray(
        np.asarray(lc1_w[:NL1], dtype=f32).reshape(NL1, 3, F, F)
        .transpose(2, 0, 1, 3).reshape(F, NL1 * 3 * F))
    w2_np = np.ascontiguousarray(
        np.asarray(lc2_w, dtype=f32).reshape(NL2, 3, F, F)
        .transpose(2, 0, 1, 3).reshape(F, NL2 * 3 * F))
    wd1_np = np.ascontiguousarray(d1_w, dtype=f32) * np.float32(0.25)
    wd2_np = np.ascontiguousarray(d2_w.reshape(ND1, 1), dtype=f32)
    cb_np = np.ascontiguousarray(conv_b.reshape(F, 1), dtype=f32)
    b1_np = np.ascontiguousarray(lc1_b[:NL1].T, dtype=f32)
    b2_np = np.ascontiguousarray(lc2_b.T, dtype=f32)
    db_np = np.ascontiguousarray(d1_b.reshape(ND1, 1), dtype=f32)
    shared = dict(wp=wp_np.astype(bf16), w1=w1_np.astype(bf16),
                  w2=w2_np.astype(bf16), wd1=wd1_np.astype(bf16),
                  wd2=wd2_np.astype(bf16),
                  cb=cb_np, b1=b1_np, b2=b2_np, db=db_np)
    xb = np.asarray(inputs, dtype=f32).astype(bf16)
    in_maps = []
    for c in range(n_cores):
        shard = np.ascontiguousarray(
            xb[c * bc:(c + 1) * bc]).reshape(bc * FEAT)
        in_maps.append(dict(shared, x=shard))
    return in_maps, (cbz, lc1z, lc2z, dbz)


def kernel(inputs, conv_w, conv_b, lc1_w, lc1_b, lc2_w, lc2_b,
           d1_w, d1_b, d2_w):
    from concourse.bass_utils import run_bass_kernel_spmd

    in_maps, flags = _prep_in_maps(
        inputs, conv_w, conv_b, lc1_w, lc1_b, lc2_w, lc2_b, d1_w, d1_b, d2_w)
    nc = _get_program(NT, flags)
    res = run_bass_kernel_spmd(nc, in_maps, list(range(N_CORES)))
    out = np.concatenate([res.results[c]["y"] for c in range(N_CORES)])
    return out.reshape(B_FULL, 1).astype(np.float32)
